# revision 14
# baseline (speedup 1.0000x reference)
"""RNN-T Joiner kernel for Trainium2 (Bass/Tile), 8-core data-parallel over batch.

out[b,t,u,v] = (enc[b,t] @ We)[v] + (pred[b,u] @ Wp)[v] + bias[v]

Per core (one batch element). The 68.2 MB output store is the roofline
(~425 GB/s per-core DMA), so the pipeline is arranged to start storing as
early as possible and never starve the DMA engines:

  - Inputs are loaded as f32r so projections run single-pass f32r matmuls
    (fp32 matmuls lower to TWO hardware passes; f32r is one).
  - Projection matmuls round-robin across independent PSUM accumulation
    chains so back-to-back matmuls never chain on the same PSUM slice.
  - pred projection is one 65-row group; the broadcast one-hot sel65
    contracts over all 65 partitions (K=65, base partition 0) for every u.
    sel65 is generated on-device (memset + gpsimd affine_select + DVE
    round-to-f32r) instead of a 4.3 MB DMA load.
  - Input loads are split across BOTH HWDGE queues (sync: predT/Wp/bias,
    scalar: encT/We) to halve issue serialization.
  - Main loop: PE broadcasts pred rows into PSUM (once per u-pair, shared
    by both t-halves). DVE adds t-half0 straight from PSUM; Scalar copies
    the pair to SBUF; gpsimd (SBUF-only engine) adds t-half1 from the copy.
  - Stores: 4-u blocks (2 MB per DMA, 16 KB descriptors), stage pools
    bufs=3 for pipelining slack.
"""

import sys

sys.path.insert(0, "/opt/trn_rl_repo")

import numpy as np

B, T, U1, D, V = 8, 256, 65, 640, 1024
KC = D // 128   # 5 contraction chunks
UBLK = 4        # u's per output DMA block: 16 blocks x 4 + tail u=64

_COMPILED = None


def _build():
    import concourse.bacc as bacc
    import concourse.tile as tile
    import concourse.mybir as mybir

    f32 = mybir.dt.float32
    f32r = mybir.dt.float32r

    nc = bacc.Bacc("TRN2", target_bir_lowering=False, debug=False, num_devices=8)

    encT = nc.dram_tensor("encT", [D, T], f32r, kind="ExternalInput")
    predT = nc.dram_tensor("predT", [D, U1], f32r, kind="ExternalInput")
    W = nc.dram_tensor("W", [2 * D, V], f32r, kind="ExternalInput")
    bias = nc.dram_tensor("bias", [1, V], f32, kind="ExternalInput")
    ones = nc.dram_tensor("ones", [1, 128], f32, kind="ExternalInput")
    out = nc.dram_tensor("out", [T, U1 * V], f32, kind="ExternalOutput")

    with tile.TileContext(nc) as tc:
        with tc.tile_pool(name="consts", bufs=1) as cp:
            sel65 = cp.tile([U1, U1 * 128], f32r, tag="sel65")
            pred_sp = cp.tile([U1, V], f32r, tag="pred_sp")
            enc_dup = []
            for tt in range(2):
                ed = cp.tile([128, 2 * V], f32, tag=f"enc_dup{tt}")
                enc_dup.append(ed)

            with tc.tile_pool(name="wpool", bufs=1) as wp:
                # ---- input loads on both HWDGE queues, dependency order ----
                predT_sb = wp.tile([128, KC * U1], f32r, tag="predT")
                nc.sync.dma_start(
                    predT_sb[:].rearrange("p (c u) -> p c u", c=KC),
                    predT[:].rearrange("(c p) u -> p c u", p=128))
                encT_sb = wp.tile([128, KC * T], f32r, tag="encT")
                nc.scalar.dma_start(
                    encT_sb[:].rearrange("p (c t) -> p c t", c=KC),
                    encT[:].rearrange("(c p) t -> p c t", p=128))
                Wp_all = wp.tile([128, KC * V], f32r, tag="Wp_all")
                nc.sync.dma_start(
                    Wp_all[:].rearrange("p (c v) -> p c v", c=KC),
                    W[D:2 * D, :].rearrange("(c p) v -> p c v", p=128))
                We_all = wp.tile([128, KC * V], f32r, tag="We_all")
                nc.scalar.dma_start(
                    We_all[:].rearrange("p (c v) -> p c v", c=KC),
                    W[0:D, :].rearrange("(c p) v -> p c v", p=128))
                Wp_sb = [Wp_all[:, c * V:(c + 1) * V] for c in range(KC)]
                We_sb = [We_all[:, c * V:(c + 1) * V] for c in range(KC)]
                bias_sb = wp.tile([1, V], f32, tag="bias")
                nc.sync.dma_start(bias_sb[:], bias[:])
                ones_sb = wp.tile([1, 128], f32, tag="ones")
                nc.sync.dma_start(ones_sb[:], ones[:])

                # sel65[r, r*128:(r+1)*128] = 1 for r<65, else 0: DVE
                # memset fills an f32 scratch with ones, gpsimd affine_select
                # zeroes off-band, DVE tensor_copy rounds into the f32r tile.
                self_ = wp.tile([U1, U1 * 128], f32, tag="self_")
                nc.vector.memset(self_[:], 1.0)
                nc.gpsimd.affine_select(
                    self_[0:U1, :].rearrange("p (s j) -> p s j", s=U1),
                    self_[0:U1, :].rearrange("p (s j) -> p s j", s=U1),
                    pattern=[[1, U1], [0, 128]],
                    compare_op=mybir.AluOpType.is_equal,
                    fill=0.0, base=0, channel_multiplier=-1)
                nc.vector.tensor_copy(sel65[:], self_[:])

                # ---- projections (f32r single-pass, chain-interleaved) ----
                with tc.tile_pool(name="spsum", bufs=1, space="PSUM") as sp:
                    ps_p = sp.tile([128, V], f32, tag="ps")
                    for c in range(KC):
                        for vt in range(2):
                            vs = slice(vt * 512, (vt + 1) * 512)
                            nc.tensor.matmul(
                                ps_p[0:U1, vs],
                                predT_sb[:, c * U1:(c + 1) * U1],
                                Wp_sb[c][:, vs],
                                start=(c == 0), stop=False)
                    for vt in range(2):
                        vs = slice(vt * 512, (vt + 1) * 512)
                        nc.tensor.matmul(
                            ps_p[0:U1, vs], ones_sb[0:1, 0:U1], bias_sb[0:1, vs],
                            start=False, stop=True)
                    nc.vector.tensor_copy(pred_sp[:], ps_p[0:U1, :])

                    # enc: c-outer (chunk consumed as its DMA lands), 4 chains
                    ps_e = []
                    for tt in range(2):
                        pe_ = sp.tile([128, V], f32, tag=f"pse{tt}")
                        ps_e.append(pe_)
                    for c in range(KC):
                        for tt in range(2):
                            for vt in range(2):
                                vs = slice(vt * 512, (vt + 1) * 512)
                                nc.tensor.matmul(
                                    ps_e[tt][:, vs],
                                    encT_sb[:, c * T + tt * 128:c * T + (tt + 1) * 128],
                                    We_sb[c][:, vs],
                                    start=(c == 0), stop=(c == KC - 1))
                    nc.vector.tensor_copy(enc_dup[0][:, 0:V], ps_e[0][:])
                    nc.scalar.copy(enc_dup[0][:, V:2 * V], ps_e[0][:])
                    nc.vector.tensor_copy(enc_dup[1][:, 0:V], ps_e[1][:])
                    nc.scalar.copy(enc_dup[1][:, V:2 * V], ps_e[1][:])

            def bcast_mm(ps_ap, u, vt):
                # one [128,512] slice of pred_b[u] broadcast to all partitions
                vs = slice(vt * 512, (vt + 1) * 512)
                nc.tensor.matmul(
                    ps_ap, sel65[0:U1, u * 128:(u + 1) * 128],
                    pred_sp[0:U1, vs], start=True, stop=True)

            def do_pair(ps, pred_sb, st0, st1, nv):
                # DVE adds t-half0 from PSUM; Scalar copies pair to SBUF;
                # gpsimd (SBUF-only) adds t-half1 from the copy.
                nc.vector.tensor_tensor(
                    st0, enc_dup[0][:, 0:nv], ps, mybir.AluOpType.add)
                nc.scalar.copy(pred_sb, ps)
                nc.gpsimd.tensor_tensor(
                    st1, enc_dup[1][:, 0:nv], pred_sb, mybir.AluOpType.add)

            # ---- main loop: broadcast + add + store ----
            with tc.tile_pool(name="outp", bufs=3) as op_, \
                 tc.tile_pool(name="pairp", bufs=3) as pp2, \
                 tc.tile_pool(name="mpsum", bufs=2, space="PSUM") as mp:
                for blk in range(16):
                    u0 = blk * UBLK
                    stage0 = op_.tile([128, UBLK * V], f32, tag="stage0")
                    stage1 = op_.tile([128, UBLK * V], f32, tag="stage1")
                    for pair in range(UBLK // 2):
                        ua = u0 + 2 * pair
                        pc = pair * 2048
                        ps = mp.tile([128, 2048], f32, tag="mps")
                        pred_sb = pp2.tile([128, 2048], f32, tag="pred_sb")
                        bcast_mm(ps[:, 0:512], ua, 0)
                        bcast_mm(ps[:, 1024:1536], ua + 1, 0)
                        bcast_mm(ps[:, 512:1024], ua, 1)
                        bcast_mm(ps[:, 1536:2048], ua + 1, 1)
                        do_pair(ps[:], pred_sb[:],
                                stage0[:, pc:pc + 2048],
                                stage1[:, pc:pc + 2048], 2048)
                    nc.sync.dma_start(
                        out[0:128, u0 * V:(u0 + UBLK) * V], stage0[:])
                    nc.sync.dma_start(
                        out[128:256, u0 * V:(u0 + UBLK) * V], stage1[:])
                # tail u = 64
                u = U1 - 1
                stage0 = op_.tile([128, UBLK * V], f32, tag="stage0")
                stage1 = op_.tile([128, UBLK * V], f32, tag="stage1")
                ps = mp.tile([128, 2048], f32, tag="mps")
                pred_sb = pp2.tile([128, 2048], f32, tag="pred_sb")
                bcast_mm(ps[:, 0:512], u, 0)
                bcast_mm(ps[:, 512:1024], u, 1)
                do_pair(ps[:, 0:V], pred_sb[:, 0:V],
                        stage0[:, 0:V], stage1[:, 0:V], V)
                nc.sync.dma_start(out[0:128, u * V:(u + 1) * V], stage0[:, 0:V])
                nc.sync.dma_start(out[128:256, u * V:(u + 1) * V], stage1[:, 0:V])

    nc.compile()
    return nc


def _get_compiled():
    global _COMPILED
    if _COMPILED is None:
        _COMPILED = _build()
    return _COMPILED


def _in_maps(encoder_out, predictor_out, W, b):
    ones = np.ones((1, 128), dtype=np.float32)
    bias = np.ascontiguousarray(b.reshape(1, V).astype(np.float32))
    Wc = np.ascontiguousarray(W.astype(np.float32))
    maps = []
    for i in range(B):
        maps.append({
            "encT": np.ascontiguousarray(encoder_out[i].T.astype(np.float32)),
            "predT": np.ascontiguousarray(predictor_out[i].T.astype(np.float32)),
            "W": Wc,
            "bias": bias,
            "ones": ones,
        })
    return maps


def run(encoder_out, predictor_out, W, b, trace=False, tmpdir=None):
    from concourse.bass_utils import run_bass_kernel_spmd

    nc = _get_compiled()
    maps = _in_maps(encoder_out, predictor_out, W, b)
    res = run_bass_kernel_spmd(
        nc, maps, list(range(B)), trace=trace,
        **({"tmpdir": tmpdir} if tmpdir else {}))
    outs = np.stack([res.results[i]["out"].reshape(T, U1, V) for i in range(B)])
    return outs, res


def kernel(encoder_out, predictor_out, W, b):
    outs, _ = run(encoder_out, predictor_out, W, b)
    return outs
